# revision 1
# baseline (speedup 1.0000x reference)
"""Causal multi-head attention (QKV proj + 16-head causal attention) on 8 TRN2 cores.

Problem: x [4, 2048, 1024], W [3072, 1024], b [3072] -> out [4, 2048, 1024].
H=16 heads, D=64. Sharding: core c = (batch b = c // 2, head-group g = c % 2);
each core computes batch b, heads g*8 .. g*8+8, producing out[b][:, g*512:(g+1)*512].
No cross-core communication needed.

Device kernel (per core), all matmuls bf16 with f32 PSUM accumulation:
  - QKV projection from host-pre-transposed xT [1024, 2048] and wT [1024, 1536]
    (bias folded in via rank-1 ones matmuls): qT, kT in feature-on-partition
    layout [128, 4, 2048]; v in token-on-partition layout with a ones column
    appended per head ([128, 16, 8, 65]) for the softmax-denominator trick.
  - Attention per (tq-chunk J of 512, head pair): S^T tiles [tk=128, tq<=512]
    = kT.T @ qT (head dim contracts on 64 partitions), exp on ScalarE over
    2-tile PSUM groups (no max subtraction -- logits are bounded by
    construction), widened causal masks (zeros before the diagonal block,
    upper-tri on it, ones after) multiply the 4 diagonal P^T tiles so that a
    full-depth accumulation is causally correct. P@v runs v-stationary:
    y^T[65, tq] = sum_i [v_i|1].T @ P^T_i, avoiding per-tile LDWEIGHTS cost;
    row 64 is the softmax denominator. Small PE transposes ([65,128] ->
    [128,65]) restore token-on-partition layout, then reciprocal +
    per-partition scalar multiply normalize, staged into [128, 512] tiles so
    the output DMA moves 2KB/partition rows at full rate.
  - Causally dead work is skipped at tile granularity and the diagonal-tile
    matmuls shrink their moving operand to the live column range.
Measured: ~316-320 us NEFF exec (from 389 us first-correct), rel err 3e-3.
"""

import numpy as np
import ml_dtypes

B, T, C = 4, 2048, 1024
H, D = 16, 64
HPC = 8            # heads per core
OC = HPC * D       # 512 output cols per core
NCORES = 8

_cache = {}


def _build_bass():
    import concourse.mybir as mybir
    import concourse.tile as tile
    from concourse import bacc
    from concourse.masks import make_identity, make_upper_triangular

    f32 = mybir.dt.float32
    bf16 = mybir.dt.bfloat16

    nc = bacc.Bacc(None)
    xt_d = nc.declare_dram_parameter("xt", [C, T], bf16, isOutput=False)
    wt_d = nc.declare_dram_parameter("wt", [C, 3 * OC], bf16, isOutput=False)
    bt_d = nc.declare_dram_parameter("bt", [1, 3 * OC], bf16, isOutput=False)
    out_d = nc.declare_dram_parameter("out", [T, OC], f32, isOutput=True)

    CT = C // 128     # 8 c-tiles
    TT = T // 128     # 16 t-tiles
    TJ = T // 512     # 4 big t-chunks

    with tile.TileContext(nc) as tc:
        with (
            tc.tile_pool(name="persist", bufs=1) as persist,
            tc.tile_pool(name="qkpsum", bufs=2, space="PSUM") as qkpsum,
            tc.tile_pool(name="spsum", bufs=2, space="PSUM") as spsum,
            tc.tile_pool(name="tpsum", bufs=2, space="PSUM") as tpsum,
            tc.tile_pool(name="pt", bufs=2) as ptpool,
            tc.tile_pool(name="ysb", bufs=1) as ypool,
            tc.tile_pool(name="small", bufs=4) as small,
        ):
            # ---- persistent SBUF tensors ----
            xt = persist.tile([128, CT, T], bf16)          # xT: [c%128, c//128, t]
            wt = persist.tile([128, CT, 3 * OC], bf16)     # wT: [c%128, c//128, o]
            bt = persist.tile([1, 3 * OC], bf16)
            ones = persist.tile([1, T], bf16)
            qT = persist.tile([128, OC // 128, T], bf16)   # q: [o%128, o//128, t]
            kT = persist.tile([128, OC // 128, T], bf16)
            vA = persist.tile([128, TT, HPC, D + 1], bf16)  # v + ones col, [t%128, t//128, h, d|1]
            # widened causal masks, one per diagonal block position jl:
            # cols < jl*128 -> 0, block jl -> upper-tri, cols after -> 1.
            # duplicated for both heads of a pair: [128, 2, 512]
            mw = persist.tile([128, 4, 512], bf16)
            iden = persist.tile([65, 65], bf16)

            for ct in range(CT):
                nc.sync.dma_start(xt[:, ct, :], xt_d[ct * 128:(ct + 1) * 128, :])
                nc.sync.dma_start(wt[:, ct, 0:2 * OC],
                                  wt_d[ct * 128:(ct + 1) * 128, 0:2 * OC])
            nc.sync.dma_start(bt[:, :], bt_d[:, :])
            for ct in range(CT):
                nc.sync.dma_start(wt[:, ct, 2 * OC:3 * OC],
                                  wt_d[ct * 128:(ct + 1) * 128, 2 * OC:3 * OC])
            nc.gpsimd.memset(ones[:, :], 1.0)
            nc.gpsimd.memset(vA[:], 1.0)                   # pre-fill ones column
            make_identity(nc, iden[:, :])
            for jl in range(4):
                if jl > 0:
                    nc.gpsimd.memset(mw[:, jl, 0:jl * 128], 0.0)
                make_upper_triangular(
                    nc, mw[:, jl, jl * 128:(jl + 1) * 128], val=1.0, diag=True)
                if jl < 3:
                    nc.gpsimd.memset(mw[:, jl, (jl + 1) * 128:512], 1.0)

            # ---- QKV projection ----
            # Q and K: out layout [o-part, t]  (o on partitions)
            for oi in range(8):                            # 4 q-tiles then 4 k-tiles
                dest = qT if oi < 4 else kT
                od = oi % 4
                for tj in range(TJ):
                    ps = qkpsum.tile([128, 512], f32, name="ps", tag="ps")
                    for ci in range(CT):
                        nc.tensor.matmul(
                            ps[:, :],
                            lhsT=wt[:, ci, oi * 128:(oi + 1) * 128],
                            rhs=xt[:, ci, tj * 512:(tj + 1) * 512],
                            start=(ci == 0), stop=False)
                    nc.tensor.matmul(
                        ps[:, :],
                        lhsT=bt[:, oi * 128:(oi + 1) * 128],
                        rhs=ones[:, tj * 512:(tj + 1) * 512],
                        start=False, stop=True)
                    nc.vector.tensor_copy(dest[:, od, tj * 512:(tj + 1) * 512], ps[:, :])
            # V: out layout [t-part, o]  (t on partitions)
            for tt in range(TT):
                ps = qkpsum.tile([128, 512], f32, name="ps", tag="ps")
                for ci in range(CT):
                    nc.tensor.matmul(
                        ps[:, :],
                        lhsT=xt[:, ci, tt * 128:(tt + 1) * 128],
                        rhs=wt[:, ci, 2 * OC:3 * OC],
                        start=(ci == 0), stop=False)
                nc.tensor.matmul(
                    ps[:, :],
                    lhsT=ones[:, tt * 128:(tt + 1) * 128],
                    rhs=bt[:, 2 * OC:3 * OC],
                    start=False, stop=True)
                for h in range(HPC):
                    nc.vector.tensor_copy(
                        vA[:, tt, h, 0:D], ps[:, h * D:(h + 1) * D])

            # ---- attention ----
            # Head-pair packed S^T (even head on array rows 0-63, odd head on
            # 64-127, adjacent issue -> concurrent sub-array execution), then
            # v-stationary P@v: y^T[65, 512] = sum_i vA_i.T @ P^T_i with the
            # widened masks zeroing the causally-invalid region, followed by
            # PE transpose back to [tq, 64|sum] layout and normalization.
            for J in range(TJ):                            # tq chunk of 512
                ysb = [ypool.tile([128, OC], f32, name=f"ysb{jl}", tag=f"ysb{jl}")
                       for jl in range(4)]
                for hp in range(4):                        # head pair
                    ni = 4 * J + 4                         # i-tiles needed (tk <= tq)
                    seq = [(i, hc) for i in range(ni) for hc in range(2)]
                    pt = ptpool.tile([128, 32, 512], bf16)
                    for g0 in range(0, 2 * ni, 2):         # exp in groups of 2 slots
                        cnt = min(2, 2 * ni - g0)
                        ps = spsum.tile([128, 2, 512], f32, name="ps", tag="ps")
                        for u in range(cnt):
                            i, hc = seq[g0 + u]
                            kp = hc * 64
                            # live tq cols: >= (i - 4J)*128 within this chunk
                            c0 = max(0, (i - 4 * J) * 128)
                            nc.tensor.matmul(
                                ps[:, u, c0:512],
                                lhsT=kT[kp:kp + 64, hp, i * 128:(i + 1) * 128],
                                rhs=qT[kp:kp + 64, hp, J * 512 + c0:(J + 1) * 512],
                                start=True, stop=True)
                        nc.scalar.activation(
                            pt[:, g0:g0 + cnt, :], ps[:, 0:cnt, :],
                            mybir.ActivationFunctionType.Exp, scale=0.125)
                    # causal masks on the 4 diagonal i-tiles (both heads at once)
                    for jl in range(4):
                        i = 4 * J + jl
                        for hc in range(2):
                            nc.vector.tensor_mul(
                                pt[:, 2 * i + hc, :],
                                pt[:, 2 * i + hc, :],
                                mw[:, jl, :])
                    for hc in range(2):
                        h = 2 * hp + hc
                        psy = qkpsum.tile([128, 512], f32, name="psy", tag="ps")
                        for i in range(ni):
                            c0 = max(0, (i - 4 * J) * 128)
                            nc.tensor.matmul(
                                psy[0:65, c0:512],
                                lhsT=vA[:, i, h, :],
                                rhs=pt[:, 2 * i + hc, c0:512],
                                start=(i == 0), stop=(i == ni - 1),
                                skip_group_check=(c0 > 0))
                        yt = small.tile([65, 512], bf16, name="yt", tag="yt")
                        nc.vector.tensor_copy(yt[:, :], psy[0:65, :])
                        for jl in range(4):
                            tps = tpsum.tile([128, 65], bf16, name="tps", tag="tps")
                            nc.tensor.transpose(
                                tps[:, :], yt[:, jl * 128:(jl + 1) * 128], iden[:, :])
                            rc = small.tile([128, 1], f32)
                            nc.vector.reciprocal(rc[:, :], tps[:, D:D + 1])
                            nc.vector.tensor_scalar_mul(
                                ysb[jl][:, h * D:(h + 1) * D], tps[:, 0:D], rc[:, :])
                for jl in range(4):
                    r0 = (4 * J + jl) * 128
                    nc.sync.dma_start(out_d[r0:r0 + 128, :], ysb[jl][:, :])

    nc.finalize()
    return nc


def _prep_inputs(x, W, b):
    """Build per-core input maps (host-side sharding + layout prep)."""
    in_maps = []
    for core in range(NCORES):
        bi, g = core // 2, core % 2
        h0 = g * HPC
        rows = []
        for sec in range(3):                      # q, k, v sections of W
            rows.append(np.arange(sec * C + h0 * D, sec * C + (h0 + HPC) * D))
        rows = np.concatenate(rows)
        Wc = W[rows, :]                           # [1536, 1024]
        bc = b[rows]                              # [1536]
        in_maps.append({
            "xt": np.ascontiguousarray(x[bi].T).astype(ml_dtypes.bfloat16),
            "wt": np.ascontiguousarray(Wc.T).astype(ml_dtypes.bfloat16),
            "bt": bc.reshape(1, -1).astype(ml_dtypes.bfloat16),
        })
    return in_maps


def kernel(x, W, b):
    from concourse.bass_utils import run_bass_kernel_spmd

    if "nc" not in _cache:
        _cache["nc"] = _build_bass()
    nc = _cache["nc"]
    in_maps = _prep_inputs(np.asarray(x), np.asarray(W), np.asarray(b))
    res = run_bass_kernel_spmd(nc, in_maps, core_ids=list(range(NCORES)))
    out = np.empty((B, T, C), dtype=np.float32)
    for core in range(NCORES):
        bi, g = core // 2, core % 2
        out[bi][:, g * OC:(g + 1) * OC] = res.results[core]["out"]
    return out



# revision 10
# speedup vs baseline: 1.4460x; 1.4460x over previous
"""Causal multi-head attention (QKV proj + 16-head causal attention) on 8 TRN2 cores.

Problem: x [4, 2048, 1024], W [3072, 1024], b [3072] -> out [4, 2048, 1024].
H=16 heads, D=64. Sharding: core c = (batch b = c // 2, head-group g = c % 2);
each core computes batch b, heads g*8 .. g*8+8, producing out[b][:, g*512:(g+1)*512].
No cross-core communication needed.

Device kernel (per core), all matmuls bf16 with f32 PSUM accumulation.
Software-pipelined so ScalarE (exp, the near-bottleneck at ~21M elements)
starts at ~6us and runs nearly continuously:
  - QKV projection chunked by 512-token groups. Q/K bias folded into the
    PSUM->SBUF cast via per-partition tensor_scalar_add (no bias matmuls);
    V bias via one rank-1 matmul per t-tile; V copied with one strided CAST
    per t-tile into vA [128, 16, 8, 65] whose ones column (softmax-denominator
    trick) is prefilled.
  - Attention per (tq-chunk J of 512, head pair hp): S^T tiles [tk=128, tq]
    = kT.T @ qT with the head dim contracting on 64 partitions and both heads
    of the pair issued adjacently (concurrent sub-array execution); exp on
    ScalarE over 2-slot PSUM groups trimmed to the causally-live column
    range; a single 128x128 upper-triangular mask multiply (on GpSimd, which
    is otherwise idle) for diagonal tiles only.
  - P@v v-stationary: y^T[65, tq] = sum_i [v_i|1].T @ P^T_i (row 64 = softmax
    denominator). Both heads' y^T are packed into one [128, 512] tile via
    partition-shifted DVE casts, then 4 128x128 PE transposes per head pair
    restore token-major layout; denominators take 4 tiny [2,128] transposes,
    one reciprocal, and per-partition tensor_scalar_mul normalization
    straight out of transpose PSUM into the output staging tile.
  - Emission keeps S^T/exp two head-pairs ahead of P@v, with QKV chunk J+1
    emitted inside attention chunk J. P^T tiles live in a 32-deep pool of
    per-i-tile [128, 2, 512] buffers whose rotation order is deadlock-free
    by construction.
"""

import numpy as np
import ml_dtypes

B, T, C = 4, 2048, 1024
H, D = 16, 64
HPC = 8            # heads per core
OC = HPC * D       # 512 output cols per core
NCORES = 8

_cache = {}


def _build_bass():
    import concourse.mybir as mybir
    import concourse.tile as tile
    from concourse import bacc
    from concourse.masks import make_identity, make_upper_triangular

    f32 = mybir.dt.float32
    bf16 = mybir.dt.bfloat16
    EXP = mybir.ActivationFunctionType.Exp

    nc = bacc.Bacc(None)
    xt_d = nc.declare_dram_parameter("xt", [C, T], bf16, isOutput=False)
    wt_d = nc.declare_dram_parameter("wt", [C, 3 * OC], bf16, isOutput=False)
    bcc_d = nc.declare_dram_parameter("bcc", [128, 8], f32, isOutput=False)
    btr_d = nc.declare_dram_parameter("btr", [1, OC], bf16, isOutput=False)
    out_d = nc.declare_dram_parameter("out", [T, OC], f32, isOutput=True)

    CT = C // 128     # 8 c-tiles
    TT = T // 128     # 16 t-tiles
    TJ = T // 512     # 4 big t-chunks

    with tile.TileContext(nc) as tc:
        with (
            tc.tile_pool(name="persist", bufs=1) as persist,
            tc.tile_pool(name="xtp", bufs=2) as xtp,
            tc.tile_pool(name="qtp", bufs=2) as qtp,
            tc.tile_pool(name="ptp", bufs=32) as ptp,
            tc.tile_pool(name="ytp", bufs=2) as ytp,
            tc.tile_pool(name="denp", bufs=2) as denp,
            tc.tile_pool(name="rcp", bufs=2) as rcp,
            tc.tile_pool(name="osbp", bufs=4) as osbp,
            tc.tile_pool(name="spsum", bufs=2, space="PSUM") as spsum,
            tc.tile_pool(name="shpsum", bufs=2, space="PSUM") as shpsum,
            tc.tile_pool(name="tpsum", bufs=2, space="PSUM") as tpsum,
        ):
            # ---- persistent SBUF tensors ----
            wt = persist.tile([128, CT, 3 * OC], bf16)     # [c%128, c//128, o]
            kT = persist.tile([128, OC // 128, T], bf16)   # [o%128, o//128, t]
            vA = persist.tile([128, TT, HPC, D + 1], bf16)  # v + ones col
            bcc = persist.tile([128, 8], f32)              # Q/K bias, col=o-tile
            btr = persist.tile([1, OC], bf16)              # V bias row
            ones = persist.tile([1, 128], bf16)
            ut = persist.tile([128, 128], bf16)            # upper-tri (incl diag)
            iden = persist.tile([128, 128], bf16)

            # early DMAs: first Q/K weight block + Q/K bias
            nc.sync.dma_start(wt[:, 0:4, 0:256], wt_d[0:512, 0:256])
            nc.sync.dma_start(wt[:, 4:8, 0:256], wt_d[512:1024, 0:256])
            nc.sync.dma_start(bcc[:, :], bcc_d[:, :])

            nc.gpsimd.memset(ones[:, :], 1.0)
            nc.gpsimd.memset(vA[:], 1.0)                   # pre-fill ones column
            make_identity(nc, iden[:, :])
            make_upper_triangular(nc, ut[:, :], val=1.0, diag=True)

            xts = [None] * TJ
            qts = [None] * TJ
            osbs = {}
            pts = {}

            def load_chunk(tj):
                xts[tj] = xtp.tile([128, CT, 512], bf16, name=f"xt{tj}", tag="xt")
                nc.sync.dma_start(xts[tj][:, 0:4, :],
                                  xt_d[0:512, tj * 512:(tj + 1) * 512])
                nc.sync.dma_start(xts[tj][:, 4:8, :],
                                  xt_d[512:1024, tj * 512:(tj + 1) * 512])
                qts[tj] = qtp.tile([128, 4, 512], bf16, name=f"qt{tj}", tag="qt")

            def qk_od(tj, g):
                """Project q and k o-tile g for token chunk tj."""
                xtt, qtt = xts[tj], qts[tj]
                for which in range(2):                     # 0 = q, 1 = k
                    ps = shpsum.tile([128, 512], f32, name="ps", tag="ps")
                    w0 = g * 256 + which * 128
                    for ci in range(CT):
                        nc.tensor.matmul(
                            ps[:, :],
                            lhsT=wt[:, ci, w0:w0 + 128],
                            rhs=xtt[:, ci, :],
                            start=(ci == 0), stop=(ci == CT - 1))
                    if which == 0:
                        nc.vector.tensor_scalar_add(
                            qtt[:, g, :], ps[:, :], bcc[:, 2 * g:2 * g + 1])
                    else:
                        nc.vector.tensor_scalar_add(
                            kT[:, g, tj * 512:(tj + 1) * 512], ps[:, :],
                            bcc[:, 2 * g + 1:2 * g + 2])

            def v_chunk(tj):
                xtt = xts[tj]
                for tl in range(4):
                    tt = tj * 4 + tl
                    ps = shpsum.tile([128, 512], f32, name="ps", tag="ps")
                    for ci in range(CT):
                        nc.tensor.matmul(
                            ps[:, :],
                            lhsT=xtt[:, ci, tl * 128:(tl + 1) * 128],
                            rhs=wt[:, ci, 2 * OC:3 * OC],
                            start=(ci == 0), stop=False)
                    nc.tensor.matmul(
                        ps[:, :], lhsT=ones[:, :], rhs=btr[:, :],
                        start=False, stop=True)
                    nc.vector.tensor_copy(vA[:, tt, :, 0:D], ps[:, :])

            def s_phase(J, hp):
                """S^T + exp + causal mask for head pair hp, tq chunk J."""
                ni = 4 * J + 4
                slots = []
                qtt = qts[J]
                for i in range(ni):
                    c0 = max(0, (i - 4 * J) * 128)
                    ptt = ptp.tile([128, 2, 512], bf16, name="pt", tag="pt")
                    slots.append(ptt)
                    sp = spsum.tile([128, 2, 512], f32, name="sp", tag="sp")
                    for hc in range(2):
                        kp = hc * 64
                        nc.tensor.matmul(
                            sp[:, hc, c0:512],
                            lhsT=kT[kp:kp + 64, hp, i * 128:(i + 1) * 128],
                            rhs=qtt[kp:kp + 64, hp, c0:512],
                            start=True, stop=True)
                    nc.scalar.activation(
                        ptt[:, 0:2, c0:512], sp[:, 0:2, c0:512],
                        EXP, scale=0.125)
                    if i >= 4 * J:                         # diagonal tile
                        for hc in range(2):
                            nc.gpsimd.tensor_mul(
                                ptt[:, hc, c0:c0 + 128],
                                ptt[:, hc, c0:c0 + 128],
                                ut[:, :])
                pts[(J, hp)] = slots

            def pv_phase(J, hp, last=False):
                """P@v, denominators, transposes, normalize for (J, hp)."""
                ni = 4 * J + 4
                slots = pts.pop((J, hp))
                if hp == 0:
                    osbs[J] = [osbp.tile([128, OC], f32, name=f"osb{J}_{jl}",
                                         tag=f"osb{jl}") for jl in range(4)]
                ytpair = ytp.tile([128, 512], bf16, name="yt", tag="yt")
                # dens land on partitions 0 (hc=0) and 32 (hc=1): partition
                # bases must be 32-aligned, and the memset keeps the [33,128]
                # transposes below off uninitialized bits
                dn = denp.tile([33, 512], bf16, name="dn", tag="dn")
                nc.gpsimd.memset(dn[:, :], 0.0)
                for hc in range(2):
                    h = 2 * hp + hc
                    psv = shpsum.tile([128, 512], f32, name="psv", tag="ps")
                    for i in range(ni):
                        c0 = max(0, (i - 4 * J) * 128)
                        nc.tensor.matmul(
                            psv[0:65, c0:512],
                            lhsT=vA[:, i, h, :],
                            rhs=slots[i][:, hc, c0:512],
                            start=(i == 0), stop=(i == ni - 1),
                            skip_group_check=(c0 > 0))
                    nc.vector.tensor_copy(
                        ytpair[hc * 64:(hc + 1) * 64, :], psv[0:64, :])
                    nc.vector.tensor_copy(
                        dn[32 * hc:32 * hc + 1, :], psv[64:65, :])
                # denominators -> token-major reciprocals [128, (hc, jl)]
                dtp = shpsum.tile([128, 4, 34], bf16, name="dtp", tag="ps")
                for jl in range(4):
                    nc.tensor.transpose(
                        dtp[:, jl, 0:33],
                        dn[:, jl * 128:(jl + 1) * 128], iden[0:33, 0:33])
                rc = rcp.tile([128, 2, 4], f32, name="rc", tag="rc")
                for hc in range(2):
                    nc.vector.reciprocal(
                        rc[:, hc, :], dtp[:, :, 32 * hc:32 * hc + 1])
                # y back to token-major + normalize
                tps = tpsum.tile([128, 4, 128], bf16, name="tps", tag="tps")
                for jl in range(4):
                    nc.tensor.transpose(
                        tps[:, jl, :], ytpair[:, jl * 128:(jl + 1) * 128],
                        iden[:, :])
                for jl in range(4):
                    for hc in range(2):
                        nc.vector.tensor_scalar_mul(
                            osbs[J][jl][:, hp * 128 + hc * 64:
                                        hp * 128 + (hc + 1) * 64],
                            tps[:, jl, hc * 64:(hc + 1) * 64],
                            rc[:, hc, jl:jl + 1])
                if last:
                    for jl in range(4):
                        r0 = (4 * J + jl) * 128
                        nc.sync.dma_start(out_d[r0:r0 + 128, :],
                                          osbs[J][jl][:, :])
                    del osbs[J]

            # ---- emission schedule ----
            # S^T/exp runs two head-pairs ahead of P@v (pt pool rotation is
            # deadlock-free at this distance with 32 bufs); QKV chunk J+1 is
            # emitted inside attention chunk J.
            load_chunk(0)

            def load_w(g):
                nc.sync.dma_start(wt[:, 0:4, g * 256:(g + 1) * 256],
                                  wt_d[0:512, g * 256:(g + 1) * 256])
                nc.sync.dma_start(wt[:, 4:8, g * 256:(g + 1) * 256],
                                  wt_d[512:1024, g * 256:(g + 1) * 256])

            qk_od(0, 0); load_w(1); s_phase(0, 0)
            qk_od(0, 1); load_w(2); s_phase(0, 1)
            load_w(3)
            nc.sync.dma_start(wt[:, 0:4, 2 * OC:3 * OC],
                              wt_d[0:512, 2 * OC:3 * OC])
            nc.sync.dma_start(wt[:, 4:8, 2 * OC:3 * OC],
                              wt_d[512:1024, 2 * OC:3 * OC])
            nc.sync.dma_start(btr[:, :], btr_d[:, :])
            qk_od(0, 2); qk_od(0, 3)
            v_chunk(0)

            for J in range(TJ):
                nj = J + 1
                pv_phase(J, 0)
                s_phase(J, 2)
                pv_phase(J, 1)
                s_phase(J, 3)
                if nj < TJ:
                    load_chunk(nj)
                    for g in range(4):
                        qk_od(nj, g)
                pv_phase(J, 2)
                if nj < TJ:
                    s_phase(nj, 0)
                pv_phase(J, 3, last=True)
                if nj < TJ:
                    s_phase(nj, 1)
                    v_chunk(nj)

    nc.finalize()
    return nc


def _prep_inputs(x, W, b):
    """Build per-core input maps (host-side sharding + layout prep)."""
    in_maps = []
    for core in range(NCORES):
        bi, g = core // 2, core % 2
        h0 = g * HPC
        # weight rows, interleaved [q0,k0,q1,k1,q2,k2,q3,k3,v] by 128-row
        # o-tiles (o-tile g covers heads h0+2g, h0+2g+1)
        blocks = []
        for gg in range(4):
            r = (h0 + 2 * gg) * D
            blocks.append(np.arange(r, r + 128))           # q o-tile gg
            blocks.append(np.arange(C + r, C + r + 128))   # k o-tile gg
        blocks.append(np.arange(2 * C + h0 * D, 2 * C + h0 * D + OC))  # v
        rows = np.concatenate(blocks)
        Wc = W[rows, :]                                    # [1536, 1024]
        bcc = np.empty((128, 8), dtype=np.float32)
        for gg in range(4):
            r = (h0 + 2 * gg) * D
            bcc[:, 2 * gg] = b[r:r + 128]
            bcc[:, 2 * gg + 1] = b[C + r:C + r + 128]
        btr = b[2 * C + h0 * D:2 * C + h0 * D + OC]
        in_maps.append({
            "xt": np.ascontiguousarray(x[bi].T).astype(ml_dtypes.bfloat16),
            "wt": np.ascontiguousarray(Wc.T).astype(ml_dtypes.bfloat16),
            "bcc": bcc,
            "btr": btr.reshape(1, -1).astype(ml_dtypes.bfloat16),
        })
    return in_maps


def kernel(x, W, b):
    from concourse.bass_utils import run_bass_kernel_spmd

    if "nc" not in _cache:
        _cache["nc"] = _build_bass()
    nc = _cache["nc"]
    in_maps = _prep_inputs(np.asarray(x), np.asarray(W), np.asarray(b))
    res = run_bass_kernel_spmd(nc, in_maps, core_ids=list(range(NCORES)))
    out = np.empty((B, T, C), dtype=np.float32)
    for core in range(NCORES):
        bi, g = core // 2, core % 2
        out[bi][:, g * OC:(g + 1) * OC] = res.results[core]["out"]
    return out
